# revision 82
# baseline (speedup 1.0000x reference)
"""GQA attention kernel for Trainium2: B=1, S=4096, D=1024, H=8 heads (hd=128).

Sharding: one head per NeuronCore (8 cores). Each core computes its head's
Q/K/V projections from the (host-transposed) full hidden states, then a
causal flash-style attention entirely on-chip, writing its [S, hd] context
slice. Host concatenates head outputs.

Fused single-pass pipeline. For each 512-wide q-chunk/seq-chunk step n:
  - finalize(n-1): softmax denominator via partition_all_reduce on the Pool
    engine's exp-sum accumulator, fused divide on DVE, ctx^T DMA'd out
    transposed [hd, S] (host transposes back - free for HW time)
  - attention(n): scoresT pair tiles [128k x 1024(2x512q)] spanning 2 PSUM
    banks, one wide exp per pair on ACT, causal mask on DVE (diagonal
    pairs, processed first so the step's dependency tail is mask-free),
    exp-sum accumulation on the otherwise-idle Pool engine, PV accumulation
    in PSUM. Diagonal k-blocks compute only their unmasked q-range
    (widths 512/384/256/256 - all >=256 so fp32r stays full rate), cutting
    ~11% of score/exp/PV work
  - projections(n+1): QT/KT/VT chunk matmuls interleaved among score pairs
    so the PE never stalls on the ACT exp stream; V transposed to natural
    layout (bf16) on the PE
  - the next step's first score pairs are pre-issued before this step's PV
    block so the ACT exp stream never drains (deepest for the proj-free
    final step).
Q/K/scores run in float32r (full-rate PE); V/probs in bf16 (also full rate,
halves transpose cost + mask-mul cost). PSUM: score pairs 2x2 + proj 2 +
PV 1 + V-transpose 1 = 8 banks. Startup DMAs are hand-scheduled across the
sync/gpsimd/scalar queues so the first projection starts ~1.5us in.
"""

from contextlib import ExitStack

import numpy as np

B, S, D = 1, 4096, 1024
H = 8
HD = D // H  # 128
P = 128
QC = 512  # q-chunk (columns per scores tile)
NDC = D // P  # 8 d-chunks
NQC = S // QC  # 8 q-chunks == seq chunks
NKB = S // P  # 32 k-blocks
SCALE = 1.0 / float(np.sqrt(HD))


def _build_mask() -> np.ndarray:
    # Mbig[kl, c] = 1.0 if kl <= c - 384 else 0.0 ; diagonal tile (ki, qc)
    # with offset t = 384 - 128*(ki - 4*qc) uses Mbig[:, t:t+512].
    kl = np.arange(P)[:, None]
    c = np.arange(896)[None, :]
    return (kl <= (c - 384)).astype(np.float32)


def _build_program():
    nc = _build_program_inner()
    nc.finalize()
    return nc


def _build_program_inner():
    from concourse import bacc, mybir, tile
    from concourse.masks import make_identity

    f32 = mybir.dt.float32
    fmm = mybir.dt.float32r
    bf16 = mybir.dt.bfloat16

    nc = bacc.Bacc("TRN2", target_bir_lowering=False, debug=True)

    xt = nc.dram_tensor("xt", [D, S], f32, kind="ExternalInput")
    wqt = nc.dram_tensor("wqt", [D, HD], f32, kind="ExternalInput")
    wkt = nc.dram_tensor("wkt", [D, HD], f32, kind="ExternalInput")
    wvt = nc.dram_tensor("wvt", [D, HD], f32, kind="ExternalInput")
    mask = nc.dram_tensor("mask", [P, 896], f32, kind="ExternalInput")
    # context emitted transposed [hd, S]; host transposes back (free for HW time)
    out = nc.dram_tensor("out", [HD, S], f32, kind="ExternalOutput")

    Exp = mybir.ActivationFunctionType.Exp

    with ExitStack() as stack:
        tc = stack.enter_context(tile.TileContext(nc))
        constp = stack.enter_context(tc.tile_pool(name="const", bufs=1))
        qkvp = stack.enter_context(tc.tile_pool(name="qkv", bufs=1))
        wp = stack.enter_context(tc.tile_pool(name="w", bufs=1))
        xtp = stack.enter_context(tc.tile_pool(name="xts", bufs=2))
        vtp = stack.enter_context(tc.tile_pool(name="vt", bufs=2))
        expp = stack.enter_context(tc.tile_pool(name="expp", bufs=23))
        esump = stack.enter_context(tc.tile_pool(name="esum", bufs=2))
        finp = stack.enter_context(tc.tile_pool(name="fin", bufs=2))
        ps_s = stack.enter_context(tc.tile_pool(name="ps_s", bufs=2, space="PSUM"))
        ps_p = stack.enter_context(tc.tile_pool(name="ps_p", bufs=2, space="PSUM"))
        ps_c = stack.enter_context(tc.tile_pool(name="ps_c", bufs=1, space="PSUM"))
        ps_m = stack.enter_context(tc.tile_pool(name="ps_m", bufs=1, space="PSUM"))

        qt_sb = qkvp.tile([P, S], fmm, tag="qt")
        kt_sb = qkvp.tile([P, S], fmm, tag="kt")
        # V natural (32 blocks [128k,128j]) in bf16: halves the PE transpose
        # cost and pairs with bf16 exp tiles in the PV matmul
        vn_sb = qkvp.tile([P, S], bf16, tag="vn")

        def dma_xt_chunk(n):
            """Load seq-chunk n of X^T as [128, 8x512]. Chunks 0/1 gate the
            pipeline start, so their d-block groups are split across idle
            queues to cut the serial DMA latency."""
            xt_n = xtp.tile([P, NDC * QC], fmm, tag="xtn", name=f"xt_n{n}")
            if n == 1:
                groups = [(0, 4, nc.sync), (4, 8, nc.gpsimd)]
            else:
                groups = [(0, NDC, nc.sync)]
            for d0, d1, eng in groups:
                eng.dma_start(
                    out=xt_n[:, d0 * QC:d1 * QC].rearrange("p (d s) -> p d s", d=d1 - d0),
                    in_=xt[d0 * P:d1 * P, n * QC:(n + 1) * QC]
                    .bitcast(fmm)
                    .rearrange("(d p) s -> p d s", p=P),
                )
            return xt_n

        # Startup DMA schedule. The first q-chain matmul needs xt0 block d0 and
        # wq's first half; spread those across the sync/gpsimd queues so both
        # land ~1us in. wk rides the scalar queue (behind the auto-inserted
        # LoadActFuncSet), wv/gpsimd, and xt1 is split sync/gpsimd so step 0's
        # interleaved proj(1) isn't starved.
        w_sb = {}
        for name in ("q", "k", "v"):
            w_sb[name] = wp.tile([P, NDC * HD], fmm, tag=f"w{name}", name=f"w{name}")

        def dma_w_half(name, dram, eng, half):
            d0, d1 = (0, NDC // 2) if half == 0 else (NDC // 2, NDC)
            eng.dma_start(
                out=w_sb[name][:, d0 * HD:d1 * HD].rearrange("p (d c) -> p d c", d=d1 - d0),
                in_=dram[d0 * P:d1 * P, :].bitcast(fmm).rearrange("(d p) c -> p d c", p=P),
            )

        xt0 = xtp.tile([P, NDC * QC], fmm, tag="xtn", name="xt_n0")

        def dma_xt0_group(d0, d1, eng):
            eng.dma_start(
                out=xt0[:, d0 * QC:d1 * QC].rearrange("p (d s) -> p d s", d=d1 - d0),
                in_=xt[d0 * P:d1 * P, 0:QC].bitcast(fmm).rearrange("(d p) s -> p d s", p=P),
            )

        # identity first: its Pool affine + DVE bf16 copy run before any
        # compute needs them, keeping the DVE queue clear for qt/kt copies
        ident = constp.tile([P, P], f32, tag="ident")
        make_identity(nc, ident[:])
        ident_bf = constp.tile([P, P], bf16, tag="ident_bf")
        nc.vector.tensor_copy(out=ident_bf[:], in_=ident[:])


        # Per-block xt0 singles alternate sync/gpsimd so the first q-chain is
        # fed at ~2 blocks per 0.8us; wq first half leads the gpsimd queue,
        # its second half leads scalar (ahead of wk, behind the act-table load).
        dma_w_half("q", wqt, nc.gpsimd, 0)
        for d in range(NDC):
            dma_xt0_group(d, d + 1, nc.sync if d % 2 == 0 else nc.gpsimd)
        dma_w_half("q", wqt, nc.scalar, 1)
        dma_w_half("k", wkt, nc.scalar, 0)
        dma_w_half("k", wkt, nc.scalar, 1)
        dma_w_half("v", wvt, nc.gpsimd, 0)
        dma_w_half("v", wvt, nc.gpsimd, 1)
        xt_tiles = {0: xt0}

        mask_sb = constp.tile([P, 896], f32, tag="mask")
        mask_bf = constp.tile([P, 896], bf16, tag="mask_bf")

        def load_mask():
            # emitted after the prologue chains: the DVE cast copy then queues
            # behind qt/kt copies instead of delaying them (masks are first
            # needed by step 0's diagonal muls, ~9us in)
            nc.scalar.dma_start(out=mask_sb[:], in_=mask[:, :])
            nc.vector.tensor_copy(out=mask_bf[:], in_=mask_sb[:])

        def proj_units(n, xt_n):
            """Yield emission units (closures) for chunk n's projections."""
            vt_n = vtp.tile([P, QC], bf16, tag="vtn", name=f"vt{n}")

            def u_q():
                ps = ps_p.tile([P, QC], f32, tag="qkvps")
                for d in range(NDC):
                    nc.tensor.matmul(
                        out=ps[:], lhsT=w_sb["q"][:, d * HD:(d + 1) * HD],
                        rhs=xt_n[:, d * QC:(d + 1) * QC],
                        start=(d == 0), stop=(d == NDC - 1),
                    )
                nc.vector.tensor_copy(out=qt_sb[:, n * QC:(n + 1) * QC], in_=ps[:])

            def u_k():
                ps = ps_p.tile([P, QC], f32, tag="qkvps")
                for d in range(NDC):
                    nc.tensor.matmul(
                        out=ps[:], lhsT=w_sb["k"][:, d * HD:(d + 1) * HD],
                        rhs=xt_n[:, d * QC:(d + 1) * QC],
                        start=(d == 0), stop=(d == NDC - 1),
                    )
                nc.vector.tensor_copy(out=kt_sb[:, n * QC:(n + 1) * QC], in_=ps[:])

            def u_v():
                ps = ps_p.tile([P, QC], f32, tag="qkvps")
                for d in range(NDC):
                    nc.tensor.matmul(
                        out=ps[:], lhsT=w_sb["v"][:, d * HD:(d + 1) * HD],
                        rhs=xt_n[:, d * QC:(d + 1) * QC],
                        start=(d == 0), stop=(d == NDC - 1),
                    )
                nc.vector.tensor_copy(out=vt_n[:], in_=ps[:])

            def u_vt():
                pt = ps_m.tile([P, QC], bf16, tag="misc")
                for j in range(QC // P):
                    nc.tensor.transpose(
                        out=pt[:, j * P:(j + 1) * P],
                        in_=vt_n[:, j * P:(j + 1) * P],
                        identity=ident_bf[:],
                    )
                nc.vector.tensor_copy(out=vn_sb[:, n * QC:(n + 1) * QC], in_=pt[:])

            return [u_q, u_k, u_v, u_vt]

        HQ = QC // 2  # 256

        def pair_geom(qc, j, h):
            """(q-offset, width) of the unmasked q-range for k-block 2j+h of
            q-chunk qc. Diagonal blocks are fully masked below the causal
            boundary, so they compute narrower (widths stay >=256 for
            full-rate fp32r):
              blocks 4qc+2 / 4qc+3 -> cols 256..511 (width 256)
              block  4qc+1         -> cols 128..511 (width 384)"""
            if j == 2 * qc + 1:
                return HQ, HQ
            if j == 2 * qc and h == 1:
                return P, QC - P
            return 0, QC

        def score_pair_unit(qc, j, esum, state, split_adds=False):
            """Pair j covers k-blocks (2j, 2j+1): 2 matmuls, 1 exp, masks,
            Pool adds. The two block slices pack side by side in the PSUM
            pair tile / exp tile. split_adds column-splits the esum adds so
            the low half completes first (lets the final partition-reduce
            piece start earlier - used for the kernel's very last pair)."""
            def u():
                sp = ps_s.tile([P, 2 * QC], f32, tag="spair")
                offs = []
                col = 0
                for h in range(2):
                    qoff, w = pair_geom(qc, j, h)
                    offs.append((col, qoff, w))
                    q0 = qc * QC + qoff
                    nc.tensor.matmul(
                        out=sp[:, col:col + w],
                        lhsT=kt_sb[:, (2 * j + h) * P:(2 * j + h + 1) * P],
                        rhs=qt_sb[:, q0:q0 + w],
                        start=True, stop=True,
                    )
                    col += w
                e = expp.tile([P, 2 * QC], bf16, tag="exp", name=f"e{qc}_{j}")
                nc.scalar.activation(out=e[:, 0:col], in_=sp[:, 0:col], func=Exp, scale=SCALE)
                for h in range(2):
                    ki = 2 * j + h
                    c0, qoff, w = offs[h]
                    if ki >= 4 * qc:  # diagonal tile: causal mask
                        t = 384 - P * (ki - 4 * qc) + qoff
                        nc.vector.tensor_mul(
                            out=e[:, c0:c0 + w],
                            in0=e[:, c0:c0 + w],
                            in1=mask_bf[:, t:t + w],
                        )
                if split_adds:
                    for half in range(2):
                        for h in range(2):
                            c0, qoff, w = offs[h]
                            hw = w // 2
                            nc.gpsimd.tensor_add(
                                out=esum[:, qoff + half * hw:qoff + (half + 1) * hw],
                                in0=esum[:, qoff + half * hw:qoff + (half + 1) * hw],
                                in1=e[:, c0 + half * hw:c0 + (half + 1) * hw],
                            )
                else:
                    for h in range(2):
                        c0, qoff, w = offs[h]
                        nc.gpsimd.tensor_add(
                            out=esum[:, qoff:qoff + w],
                            in0=esum[:, qoff:qoff + w],
                            in1=e[:, c0:c0 + w],
                        )
                state["exps"][j] = e
            return u

        def finalize(qc, c_ps, esum, npiece=1):
            """Z (Pool partition-reduce) -> recip -> normalize -> DMA out.

            npiece>1 splits into column pieces so Pool/DVE/DMA pipeline —
            used for the last step, where this chain is the kernel tail."""
            from concourse import bass_isa

            zrep = finp.tile([P, QC], f32, tag="zrep")
            rz = finp.tile([P, QC], f32, tag="rz")
            cs = finp.tile([P, QC], f32, tag="cs")
            w = QC // npiece
            dma_engs = [nc.sync, nc.scalar, nc.gpsimd, nc.sync]
            for i in range(npiece):
                sl = slice(i * w, (i + 1) * w)
                nc.gpsimd.partition_all_reduce(
                    out_ap=zrep[:, sl], in_ap=esum[:, sl], channels=P,
                    reduce_op=bass_isa.ReduceOp.add,
                )
                nc.vector.reciprocal(out=rz[:, sl], in_=zrep[:, sl])
                nc.vector.tensor_mul(out=cs[:, sl], in0=c_ps[:, sl], in1=rz[:, sl])
                dma_engs[i % len(dma_engs)].dma_start(
                    out=out[:, qc * QC + i * w:qc * QC + (i + 1) * w], in_=cs[:, sl]
                )


        def make_score_units(n):
            """esum tile + score-pair units for step n, diagonal pairs first:
            their mask-muls then overlap later exps, and the final pair of the
            step is mask-free, which shortens the exp->esum->finalize tail."""
            esum = esump.tile([P, QC], f32, tag="esum", name=f"esum{n}")
            nc.gpsimd.memset(esum[:], 0.0)
            state = {"exps": {}}
            npair = 2 * n + 2
            pair_order = list(range(npair))
            pair_order = pair_order[-2:] + pair_order[:-2]
            units = [
                score_pair_unit(
                    n, j, esum, state,
                    split_adds=(n == NQC - 1 and idx == len(pair_order) - 1),
                )
                for idx, j in enumerate(pair_order)
            ]
            return esum, state, units

        # pairs of the final step to pre-issue during the prior step, so the
        # ACT exp stream (which paces the proj-free last step) starts early
        PRE_LAST = 9

        prev = None  # (qc, c_ps, esum) awaiting finalize
        step_state = {}

        # ---------------- prologue ----------------
        # Emit only Q/K chains of chunk 0; V chain + transpose join step 0's
        # interleave (wv arrives later on the gpsimd queue). The V chain uses
        # the misc PSUM bank so it need not wait for a ps_p slot to drain.
        # The q copy rides the ACT queue so it runs in parallel with the kt
        # copy on DVE - the first score pair needs both.
        for wname, dst, copy_eng in (
            ("q", qt_sb, "scalar"), ("k", kt_sb, "vector"),
        ):
            ps0 = ps_p.tile([P, QC], f32, tag="qkvps", name=f"p0{wname}")
            for d in range(NDC):
                nc.tensor.matmul(
                    out=ps0[:], lhsT=w_sb[wname][:, d * HD:(d + 1) * HD],
                    rhs=xt0[:, d * QC:(d + 1) * QC],
                    start=(d == 0), stop=(d == NDC - 1),
                )
            if copy_eng == "scalar":
                nc.scalar.copy(out=dst[:, 0:QC], in_=ps0[:])
            else:
                nc.vector.tensor_copy(out=dst[:, 0:QC], in_=ps0[:])
        load_mask()
        vt_0 = vtp.tile([P, QC], bf16, tag="vtn", name="vt0c")

        def u_v0():
            ps = ps_m.tile([P, QC], f32, tag="misc")
            for d in range(NDC):
                nc.tensor.matmul(
                    out=ps[:], lhsT=w_sb["v"][:, d * HD:(d + 1) * HD],
                    rhs=xt0[:, d * QC:(d + 1) * QC],
                    start=(d == 0), stop=(d == NDC - 1),
                )
            nc.vector.tensor_copy(out=vt_0[:], in_=ps[:])

        def u_vt0():
            pt = ps_m.tile([P, QC], bf16, tag="misc")
            for j in range(QC // P):
                nc.tensor.transpose(
                    out=pt[:, j * P:(j + 1) * P],
                    in_=vt_0[:, j * P:(j + 1) * P],
                    identity=ident_bf[:],
                )
            nc.vector.tensor_copy(out=vn_sb[:, 0:QC], in_=pt[:])

        carry_units = [u_v0, u_vt0]

        for n in range(NQC):
            # prefetch next xt chunk
            if n + 1 < NQC:
                xt_tiles[n + 1] = dma_xt_chunk(n + 1)

            # finalize previous step: Z, recip, normalize, DMA out
            if prev is not None:
                pqc, pc_ps, pesum = prev
                finalize(pqc, pc_ps, pesum)

            # attention scores for step n, interleaved with proj chunk n+1
            if n in step_state:
                esum, state, units = step_state.pop(n)
            else:
                esum, state, units = make_score_units(n)
            # step 0: the carried V-chain of chunk 0 goes FIRST - it is ready
            # (wv just landed) while the first score pair still waits on the
            # qt/kt PSUM->SBUF copies
            head = carry_units
            carry_units = []
            pu = proj_units(n + 1, xt_tiles[n + 1]) if n + 1 < NQC else []
            npair = 2 * n + 2
            # interleave: spread proj units among score pairs
            merged = list(head)
            if pu:
                k = max(1, npair // (len(pu) + 1))
                pi = 0
                for i, su in enumerate(units):
                    merged.append(su)
                    if (i + 1) % k == 0 and pi < len(pu):
                        merged.append(pu[pi])
                        pi += 1
                merged.extend(pu[pi:])
            else:
                merged = list(head) + units
            if n + 1 < NQC:
                # prepare next step's units; pre-issue the first pairs so the
                # ACT exp stream keeps running through this step's PV block
                pre = PRE_LAST if n + 1 == NQC - 1 else (5 if n + 1 == NQC - 2 else 3)
                nesum, nstate, nunits = make_score_units(n + 1)
                merged.extend(nunits[:pre])
                step_state[n + 1] = (nesum, nstate, nunits[pre:])
            for u in merged:
                u()

            # PV accumulation for step n. Narrowed diagonal blocks accumulate
            # onto their unmasked q-range only; pair 0 h=0 is always full
            # width, so start=True zeroes the whole tile.
            npair = 2 * n + 2
            c_ps = ps_c.tile([P, QC], f32, tag="cps")
            for j in range(npair):
                e = state["exps"][j]
                col = 0
                for h in range(2):
                    ki = 2 * j + h
                    qoff, w = pair_geom(n, j, h)
                    nc.tensor.matmul(
                        out=c_ps[:, qoff:qoff + w],
                        lhsT=vn_sb[:, ki * P:(ki + 1) * P],
                        rhs=e[:, col:col + w],
                        start=(j == 0 and h == 0),
                        stop=(j == npair - 1 and h == 1),
                    )
                    col += w
            prev = (n, c_ps, esum)

        # epilogue: finalize last step, split into pieces to shorten the tail
        pqc, pc_ps, pesum = prev
        finalize(pqc, pc_ps, pesum, npiece=2)

    return nc


_NC_CACHE = None


def _get_nc():
    global _NC_CACHE
    if _NC_CACHE is None:
        _NC_CACHE = _build_program()
    return _NC_CACHE


def kernel(hidden_states, Wq, Wk, Wv, trace=False, **trace_kwargs):
    from concourse.bass_utils import run_bass_kernel_spmd

    x = np.ascontiguousarray(np.asarray(hidden_states, dtype=np.float32)[0])  # [S, D]
    xt = np.ascontiguousarray(x.T)  # [D, S]
    mask = _build_mask()
    in_maps = []
    for h in range(H):
        in_maps.append({
            "xt": xt,
            "wqt": np.ascontiguousarray(np.asarray(Wq[h * HD:(h + 1) * HD, :], dtype=np.float32).T),
            "wkt": np.ascontiguousarray(np.asarray(Wk[h * HD:(h + 1) * HD, :], dtype=np.float32).T),
            "wvt": np.ascontiguousarray(np.asarray(Wv[h * HD:(h + 1) * HD, :], dtype=np.float32).T),
            "mask": mask,
        })

    nc = _get_nc()
    res = run_bass_kernel_spmd(
        nc, in_maps, core_ids=list(range(H)), trace=trace, **trace_kwargs
    )
    ctx = np.empty((B, S, D), dtype=np.float32)
    for h in range(H):
        ctx[0, :, h * HD:(h + 1) * HD] = res.results[h]["out"].T
    if trace:
        return ctx, res
    return ctx



# revision 84
# speedup vs baseline: 1.0058x; 1.0058x over previous
"""GQA attention kernel for Trainium2: B=1, S=4096, D=1024, H=8 heads (hd=128).

Sharding: one head per NeuronCore (8 cores). Each core computes its head's
Q/K/V projections from the (host-transposed) full hidden states, then a
causal flash-style attention entirely on-chip, writing its [S, hd] context
slice. Host concatenates head outputs.

Fused single-pass pipeline. For each 512-wide q-chunk/seq-chunk step n:
  - finalize(n-1): softmax denominator via partition_all_reduce on the Pool
    engine's exp-sum accumulator, fused divide on DVE, ctx^T DMA'd out
    transposed [hd, S] (host transposes back - free for HW time)
  - attention(n): scoresT pair tiles [128k x 1024(2x512q)] spanning 2 PSUM
    banks, one wide exp per pair on ACT, causal mask on DVE (diagonal
    pairs, processed first so the step's dependency tail is mask-free),
    exp-sum accumulation on the otherwise-idle Pool engine, PV accumulation
    in PSUM. Diagonal k-blocks compute only their unmasked q-range
    (widths 512/384/256/128), cutting ~12% of score/exp/PV work
  - projections(n+1): QT/KT/VT chunk matmuls interleaved among score pairs
    so the PE never stalls on the ACT exp stream; V transposed to natural
    layout (bf16) on the PE
  - the next step's first score pairs are pre-issued before this step's PV
    block so the ACT exp stream never drains (deepest for the proj-free
    final step).
Projections contract in float32r (full-rate PE); Q/K/V/probs are stored
bf16 (same matmul rate, no sub-256-width penalty, halves copies/masks). PSUM: score pairs 2x2 + proj 2 +
PV 1 + V-transpose 1 = 8 banks. Startup DMAs are hand-scheduled across the
sync/gpsimd/scalar queues so the first projection starts ~1.5us in.
"""

from contextlib import ExitStack

import numpy as np

B, S, D = 1, 4096, 1024
H = 8
HD = D // H  # 128
P = 128
QC = 512  # q-chunk (columns per scores tile)
NDC = D // P  # 8 d-chunks
NQC = S // QC  # 8 q-chunks == seq chunks
NKB = S // P  # 32 k-blocks
SCALE = 1.0 / float(np.sqrt(HD))


def _build_mask() -> np.ndarray:
    # Mbig[kl, c] = 1.0 if kl <= c - 384 else 0.0 ; diagonal tile (ki, qc)
    # with offset t = 384 - 128*(ki - 4*qc) uses Mbig[:, t:t+512].
    kl = np.arange(P)[:, None]
    c = np.arange(896)[None, :]
    return (kl <= (c - 384)).astype(np.float32)


def _build_program():
    nc = _build_program_inner()
    nc.finalize()
    return nc


def _build_program_inner():
    from concourse import bacc, mybir, tile
    from concourse.masks import make_identity

    f32 = mybir.dt.float32
    fmm = mybir.dt.float32r
    bf16 = mybir.dt.bfloat16

    nc = bacc.Bacc("TRN2", target_bir_lowering=False, debug=True)

    xt = nc.dram_tensor("xt", [D, S], f32, kind="ExternalInput")
    wqt = nc.dram_tensor("wqt", [D, HD], f32, kind="ExternalInput")
    wkt = nc.dram_tensor("wkt", [D, HD], f32, kind="ExternalInput")
    wvt = nc.dram_tensor("wvt", [D, HD], f32, kind="ExternalInput")
    mask = nc.dram_tensor("mask", [P, 896], f32, kind="ExternalInput")
    # context emitted transposed [hd, S]; host transposes back (free for HW time)
    out = nc.dram_tensor("out", [HD, S], f32, kind="ExternalOutput")

    Exp = mybir.ActivationFunctionType.Exp

    with ExitStack() as stack:
        tc = stack.enter_context(tile.TileContext(nc))
        constp = stack.enter_context(tc.tile_pool(name="const", bufs=1))
        qkvp = stack.enter_context(tc.tile_pool(name="qkv", bufs=1))
        wp = stack.enter_context(tc.tile_pool(name="w", bufs=1))
        xtp = stack.enter_context(tc.tile_pool(name="xts", bufs=2))
        vtp = stack.enter_context(tc.tile_pool(name="vt", bufs=2))
        expp = stack.enter_context(tc.tile_pool(name="expp", bufs=23))
        esump = stack.enter_context(tc.tile_pool(name="esum", bufs=2))
        finp = stack.enter_context(tc.tile_pool(name="fin", bufs=2))
        ps_s = stack.enter_context(tc.tile_pool(name="ps_s", bufs=2, space="PSUM"))
        ps_p = stack.enter_context(tc.tile_pool(name="ps_p", bufs=2, space="PSUM"))
        ps_c = stack.enter_context(tc.tile_pool(name="ps_c", bufs=1, space="PSUM"))
        ps_m = stack.enter_context(tc.tile_pool(name="ps_m", bufs=1, space="PSUM"))

        # Q/K in bf16: same PE rate for the score matmuls, but bf16 has no
        # sub-256-width fp32r penalty, letting diagonal blocks narrow fully
        qt_sb = qkvp.tile([P, S], bf16, tag="qt")
        kt_sb = qkvp.tile([P, S], bf16, tag="kt")
        # V natural (32 blocks [128k,128j]) in bf16: halves the PE transpose
        # cost and pairs with bf16 exp tiles in the PV matmul
        vn_sb = qkvp.tile([P, S], bf16, tag="vn")

        def dma_xt_chunk(n):
            """Load seq-chunk n of X^T as [128, 8x512]. Chunks 0/1 gate the
            pipeline start, so their d-block groups are split across idle
            queues to cut the serial DMA latency."""
            xt_n = xtp.tile([P, NDC * QC], fmm, tag="xtn", name=f"xt_n{n}")
            if n == 1:
                groups = [(0, 4, nc.sync), (4, 8, nc.gpsimd)]
            else:
                groups = [(0, NDC, nc.sync)]
            for d0, d1, eng in groups:
                eng.dma_start(
                    out=xt_n[:, d0 * QC:d1 * QC].rearrange("p (d s) -> p d s", d=d1 - d0),
                    in_=xt[d0 * P:d1 * P, n * QC:(n + 1) * QC]
                    .bitcast(fmm)
                    .rearrange("(d p) s -> p d s", p=P),
                )
            return xt_n

        # Startup DMA schedule. The first q-chain matmul needs xt0 block d0 and
        # wq's first half; spread those across the sync/gpsimd queues so both
        # land ~1us in. wk rides the scalar queue (behind the auto-inserted
        # LoadActFuncSet), wv/gpsimd, and xt1 is split sync/gpsimd so step 0's
        # interleaved proj(1) isn't starved.
        w_sb = {}
        for name in ("q", "k", "v"):
            w_sb[name] = wp.tile([P, NDC * HD], fmm, tag=f"w{name}", name=f"w{name}")

        def dma_w_half(name, dram, eng, half):
            d0, d1 = (0, NDC // 2) if half == 0 else (NDC // 2, NDC)
            eng.dma_start(
                out=w_sb[name][:, d0 * HD:d1 * HD].rearrange("p (d c) -> p d c", d=d1 - d0),
                in_=dram[d0 * P:d1 * P, :].bitcast(fmm).rearrange("(d p) c -> p d c", p=P),
            )

        xt0 = xtp.tile([P, NDC * QC], fmm, tag="xtn", name="xt_n0")

        def dma_xt0_group(d0, d1, eng):
            eng.dma_start(
                out=xt0[:, d0 * QC:d1 * QC].rearrange("p (d s) -> p d s", d=d1 - d0),
                in_=xt[d0 * P:d1 * P, 0:QC].bitcast(fmm).rearrange("(d p) s -> p d s", p=P),
            )

        # identity first: its Pool affine + DVE bf16 copy run before any
        # compute needs them, keeping the DVE queue clear for qt/kt copies
        ident = constp.tile([P, P], f32, tag="ident")
        make_identity(nc, ident[:])
        ident_bf = constp.tile([P, P], bf16, tag="ident_bf")
        nc.vector.tensor_copy(out=ident_bf[:], in_=ident[:])


        # Per-block xt0 singles alternate sync/gpsimd so the first q-chain is
        # fed at ~2 blocks per 0.8us; wq first half leads the gpsimd queue,
        # its second half leads scalar (ahead of wk, behind the act-table load).
        dma_w_half("q", wqt, nc.gpsimd, 0)
        for d in range(NDC):
            dma_xt0_group(d, d + 1, nc.sync if d % 2 == 0 else nc.gpsimd)
        dma_w_half("q", wqt, nc.scalar, 1)
        dma_w_half("k", wkt, nc.scalar, 0)
        dma_w_half("k", wkt, nc.scalar, 1)
        dma_w_half("v", wvt, nc.gpsimd, 0)
        dma_w_half("v", wvt, nc.gpsimd, 1)
        xt_tiles = {0: xt0}

        mask_sb = constp.tile([P, 896], f32, tag="mask")
        mask_bf = constp.tile([P, 896], bf16, tag="mask_bf")

        def load_mask():
            # emitted after the prologue chains: the DVE cast copy then queues
            # behind qt/kt copies instead of delaying them (masks are first
            # needed by step 0's diagonal muls, ~9us in)
            nc.scalar.dma_start(out=mask_sb[:], in_=mask[:, :])
            nc.vector.tensor_copy(out=mask_bf[:], in_=mask_sb[:])

        def proj_units(n, xt_n):
            """Yield emission units (closures) for chunk n's projections."""
            vt_n = vtp.tile([P, QC], bf16, tag="vtn", name=f"vt{n}")

            def u_q():
                ps = ps_p.tile([P, QC], f32, tag="qkvps")
                for d in range(NDC):
                    nc.tensor.matmul(
                        out=ps[:], lhsT=w_sb["q"][:, d * HD:(d + 1) * HD],
                        rhs=xt_n[:, d * QC:(d + 1) * QC],
                        start=(d == 0), stop=(d == NDC - 1),
                    )
                nc.vector.tensor_copy(out=qt_sb[:, n * QC:(n + 1) * QC], in_=ps[:])

            def u_k():
                ps = ps_p.tile([P, QC], f32, tag="qkvps")
                for d in range(NDC):
                    nc.tensor.matmul(
                        out=ps[:], lhsT=w_sb["k"][:, d * HD:(d + 1) * HD],
                        rhs=xt_n[:, d * QC:(d + 1) * QC],
                        start=(d == 0), stop=(d == NDC - 1),
                    )
                nc.vector.tensor_copy(out=kt_sb[:, n * QC:(n + 1) * QC], in_=ps[:])

            def u_v():
                ps = ps_p.tile([P, QC], f32, tag="qkvps")
                for d in range(NDC):
                    nc.tensor.matmul(
                        out=ps[:], lhsT=w_sb["v"][:, d * HD:(d + 1) * HD],
                        rhs=xt_n[:, d * QC:(d + 1) * QC],
                        start=(d == 0), stop=(d == NDC - 1),
                    )
                nc.vector.tensor_copy(out=vt_n[:], in_=ps[:])

            def u_vt():
                pt = ps_m.tile([P, QC], bf16, tag="misc")
                for j in range(QC // P):
                    nc.tensor.transpose(
                        out=pt[:, j * P:(j + 1) * P],
                        in_=vt_n[:, j * P:(j + 1) * P],
                        identity=ident_bf[:],
                    )
                nc.vector.tensor_copy(out=vn_sb[:, n * QC:(n + 1) * QC], in_=pt[:])

            return [u_q, u_k, u_v, u_vt]

        HQ = QC // 2  # 256

        def pair_geom(qc, j, h):
            """(q-offset, width) of the unmasked q-range for k-block 2j+h of
            q-chunk qc. Diagonal blocks are fully masked below the causal
            boundary, so they compute narrower (widths stay >=256 for
            full-rate fp32r):
              block 4qc+1 -> cols 128..511 (width 384)
              block 4qc+2 -> cols 256..511 (width 256)
              block 4qc+3 -> cols 384..511 (width 128; bf16 matmuls keep
              full rate below width 256)"""
            if j == 2 * qc + 1:
                return (HQ, HQ) if h == 0 else (HQ + P, HQ - P)
            if j == 2 * qc and h == 1:
                return P, QC - P
            return 0, QC

        def score_pair_unit(qc, j, esum, state, split_adds=False):
            """Pair j covers k-blocks (2j, 2j+1): 2 matmuls, 1 exp, masks,
            Pool adds. The two block slices pack side by side in the PSUM
            pair tile / exp tile. split_adds column-splits the esum adds so
            the low half completes first (lets the final partition-reduce
            piece start earlier - used for the kernel's very last pair)."""
            def u():
                sp = ps_s.tile([P, 2 * QC], f32, tag="spair")
                offs = []
                col = 0
                for h in range(2):
                    qoff, w = pair_geom(qc, j, h)
                    offs.append((col, qoff, w))
                    q0 = qc * QC + qoff
                    nc.tensor.matmul(
                        out=sp[:, col:col + w],
                        lhsT=kt_sb[:, (2 * j + h) * P:(2 * j + h + 1) * P],
                        rhs=qt_sb[:, q0:q0 + w],
                        start=True, stop=True,
                    )
                    col += w
                e = expp.tile([P, 2 * QC], bf16, tag="exp", name=f"e{qc}_{j}")
                nc.scalar.activation(out=e[:, 0:col], in_=sp[:, 0:col], func=Exp, scale=SCALE)
                for h in range(2):
                    ki = 2 * j + h
                    c0, qoff, w = offs[h]
                    if ki >= 4 * qc:  # diagonal tile: causal mask
                        t = 384 - P * (ki - 4 * qc) + qoff
                        nc.vector.tensor_mul(
                            out=e[:, c0:c0 + w],
                            in0=e[:, c0:c0 + w],
                            in1=mask_bf[:, t:t + w],
                        )
                if split_adds:
                    for half in range(2):
                        for h in range(2):
                            c0, qoff, w = offs[h]
                            hw = w // 2
                            nc.gpsimd.tensor_add(
                                out=esum[:, qoff + half * hw:qoff + (half + 1) * hw],
                                in0=esum[:, qoff + half * hw:qoff + (half + 1) * hw],
                                in1=e[:, c0 + half * hw:c0 + (half + 1) * hw],
                            )
                else:
                    for h in range(2):
                        c0, qoff, w = offs[h]
                        nc.gpsimd.tensor_add(
                            out=esum[:, qoff:qoff + w],
                            in0=esum[:, qoff:qoff + w],
                            in1=e[:, c0:c0 + w],
                        )
                state["exps"][j] = e
            return u

        def finalize(qc, c_ps, esum, npiece=1):
            """Z (Pool partition-reduce) -> recip -> normalize -> DMA out.

            npiece>1 splits into column pieces so Pool/DVE/DMA pipeline —
            used for the last step, where this chain is the kernel tail."""
            from concourse import bass_isa

            zrep = finp.tile([P, QC], f32, tag="zrep")
            rz = finp.tile([P, QC], f32, tag="rz")
            cs = finp.tile([P, QC], f32, tag="cs")
            w = QC // npiece
            dma_engs = [nc.sync, nc.scalar, nc.gpsimd, nc.sync]
            for i in range(npiece):
                sl = slice(i * w, (i + 1) * w)
                nc.gpsimd.partition_all_reduce(
                    out_ap=zrep[:, sl], in_ap=esum[:, sl], channels=P,
                    reduce_op=bass_isa.ReduceOp.add,
                )
                nc.vector.reciprocal(out=rz[:, sl], in_=zrep[:, sl])
                nc.vector.tensor_mul(out=cs[:, sl], in0=c_ps[:, sl], in1=rz[:, sl])
                dma_engs[i % len(dma_engs)].dma_start(
                    out=out[:, qc * QC + i * w:qc * QC + (i + 1) * w], in_=cs[:, sl]
                )


        def make_score_units(n):
            """esum tile + score-pair units for step n, diagonal pairs first:
            their mask-muls then overlap later exps, and the final pair of the
            step is mask-free, which shortens the exp->esum->finalize tail."""
            esum = esump.tile([P, QC], f32, tag="esum", name=f"esum{n}")
            nc.gpsimd.memset(esum[:], 0.0)
            state = {"exps": {}}
            npair = 2 * n + 2
            pair_order = list(range(npair))
            pair_order = pair_order[-2:] + pair_order[:-2]
            units = [
                score_pair_unit(
                    n, j, esum, state,
                    split_adds=(n == NQC - 1 and idx == len(pair_order) - 1),
                )
                for idx, j in enumerate(pair_order)
            ]
            return esum, state, units

        # pairs of the final step to pre-issue during the prior step, so the
        # ACT exp stream (which paces the proj-free last step) starts early
        PRE_LAST = 9

        prev = None  # (qc, c_ps, esum) awaiting finalize
        step_state = {}

        # ---------------- prologue ----------------
        # Emit only Q/K chains of chunk 0; V chain + transpose join step 0's
        # interleave (wv arrives later on the gpsimd queue). The V chain uses
        # the misc PSUM bank so it need not wait for a ps_p slot to drain.
        # The q copy rides the ACT queue so it runs in parallel with the kt
        # copy on DVE - the first score pair needs both.
        for wname, dst, copy_eng in (
            ("q", qt_sb, "scalar"), ("k", kt_sb, "vector"),
        ):
            ps0 = ps_p.tile([P, QC], f32, tag="qkvps", name=f"p0{wname}")
            for d in range(NDC):
                nc.tensor.matmul(
                    out=ps0[:], lhsT=w_sb[wname][:, d * HD:(d + 1) * HD],
                    rhs=xt0[:, d * QC:(d + 1) * QC],
                    start=(d == 0), stop=(d == NDC - 1),
                )
            if copy_eng == "scalar":
                nc.scalar.copy(out=dst[:, 0:QC], in_=ps0[:])
            else:
                nc.vector.tensor_copy(out=dst[:, 0:QC], in_=ps0[:])
        load_mask()
        vt_0 = vtp.tile([P, QC], bf16, tag="vtn", name="vt0c")

        def u_v0():
            ps = ps_m.tile([P, QC], f32, tag="misc")
            for d in range(NDC):
                nc.tensor.matmul(
                    out=ps[:], lhsT=w_sb["v"][:, d * HD:(d + 1) * HD],
                    rhs=xt0[:, d * QC:(d + 1) * QC],
                    start=(d == 0), stop=(d == NDC - 1),
                )
            nc.vector.tensor_copy(out=vt_0[:], in_=ps[:])

        def u_vt0():
            pt = ps_m.tile([P, QC], bf16, tag="misc")
            for j in range(QC // P):
                nc.tensor.transpose(
                    out=pt[:, j * P:(j + 1) * P],
                    in_=vt_0[:, j * P:(j + 1) * P],
                    identity=ident_bf[:],
                )
            nc.vector.tensor_copy(out=vn_sb[:, 0:QC], in_=pt[:])

        carry_units = [u_v0, u_vt0]

        for n in range(NQC):
            # prefetch next xt chunk
            if n + 1 < NQC:
                xt_tiles[n + 1] = dma_xt_chunk(n + 1)

            # finalize previous step: Z, recip, normalize, DMA out
            if prev is not None:
                pqc, pc_ps, pesum = prev
                finalize(pqc, pc_ps, pesum)

            # attention scores for step n, interleaved with proj chunk n+1
            if n in step_state:
                esum, state, units = step_state.pop(n)
            else:
                esum, state, units = make_score_units(n)
            # step 0: the carried V-chain of chunk 0 goes FIRST - it is ready
            # (wv just landed) while the first score pair still waits on the
            # qt/kt PSUM->SBUF copies
            head = carry_units
            carry_units = []
            pu = proj_units(n + 1, xt_tiles[n + 1]) if n + 1 < NQC else []
            npair = 2 * n + 2
            # interleave: spread proj units among score pairs
            merged = list(head)
            if pu:
                k = max(1, npair // (len(pu) + 1))
                pi = 0
                for i, su in enumerate(units):
                    merged.append(su)
                    if (i + 1) % k == 0 and pi < len(pu):
                        merged.append(pu[pi])
                        pi += 1
                merged.extend(pu[pi:])
            else:
                merged = list(head) + units
            if n + 1 < NQC:
                # prepare next step's units; pre-issue the first pairs so the
                # ACT exp stream keeps running through this step's PV block
                pre = PRE_LAST if n + 1 == NQC - 1 else (5 if n + 1 == NQC - 2 else 3)
                nesum, nstate, nunits = make_score_units(n + 1)
                merged.extend(nunits[:pre])
                step_state[n + 1] = (nesum, nstate, nunits[pre:])
            for u in merged:
                u()

            # PV accumulation for step n. Narrowed diagonal blocks accumulate
            # onto their unmasked q-range only; pair 0 h=0 is always full
            # width, so start=True zeroes the whole tile.
            npair = 2 * n + 2
            c_ps = ps_c.tile([P, QC], f32, tag="cps")
            for j in range(npair):
                e = state["exps"][j]
                col = 0
                for h in range(2):
                    ki = 2 * j + h
                    qoff, w = pair_geom(n, j, h)
                    nc.tensor.matmul(
                        out=c_ps[:, qoff:qoff + w],
                        lhsT=vn_sb[:, ki * P:(ki + 1) * P],
                        rhs=e[:, col:col + w],
                        start=(j == 0 and h == 0),
                        stop=(j == npair - 1 and h == 1),
                    )
                    col += w
            prev = (n, c_ps, esum)

        # epilogue: finalize last step, split into pieces to shorten the tail
        pqc, pc_ps, pesum = prev
        finalize(pqc, pc_ps, pesum, npiece=2)

    return nc


_NC_CACHE = None


def _get_nc():
    global _NC_CACHE
    if _NC_CACHE is None:
        _NC_CACHE = _build_program()
    return _NC_CACHE


def kernel(hidden_states, Wq, Wk, Wv, trace=False, **trace_kwargs):
    from concourse.bass_utils import run_bass_kernel_spmd

    x = np.ascontiguousarray(np.asarray(hidden_states, dtype=np.float32)[0])  # [S, D]
    xt = np.ascontiguousarray(x.T)  # [D, S]
    mask = _build_mask()
    in_maps = []
    for h in range(H):
        in_maps.append({
            "xt": xt,
            "wqt": np.ascontiguousarray(np.asarray(Wq[h * HD:(h + 1) * HD, :], dtype=np.float32).T),
            "wkt": np.ascontiguousarray(np.asarray(Wk[h * HD:(h + 1) * HD, :], dtype=np.float32).T),
            "wvt": np.ascontiguousarray(np.asarray(Wv[h * HD:(h + 1) * HD, :], dtype=np.float32).T),
            "mask": mask,
        })

    nc = _get_nc()
    res = run_bass_kernel_spmd(
        nc, in_maps, core_ids=list(range(H)), trace=trace, **trace_kwargs
    )
    ctx = np.empty((B, S, D), dtype=np.float32)
    for h in range(H):
        ctx[0, :, h * HD:(h + 1) * HD] = res.results[h]["out"].T
    if trace:
        return ctx, res
    return ctx

